# revision 12
# baseline (speedup 1.0000x reference)
"""Trainium2 Bass kernel for ContourIntegrationLayer.

Reference computation (per batch element, fp32):
    conv = depthwise_conv2d(x, kernel, 5x5, SAME zero-pad)   # per-channel
    y    = (conv * alpha + bias) * x + x

Formulation: banded matmul over the ROW dimension.  Per channel c, x is
laid out with input rows on partitions and (img, col) on the free dim:
    xt[r, i, cp]  (112 part, img, 116 padded cols), fp16
The 5x5 depthwise conv becomes 5 accumulated PE matmul chains (one per
kernel column dc):
    out[h, (i,w)] = sum_dc sum_r  Wdc[r, h] * xt[r, i, w+dc]
where Wdc[r, h] = alpha*k[r-h+2, dc, c] for |r-h|<=2 (banded, built on
host, fp16).  K=112, M=112, N=448 per matmul -> 1 cycle/col in fp16:
2240 PE cycles per channel-image-set of 4 (the PE does 112 parallel
MACs/cycle along the contraction dim instead of a diag formulation's
1/lane).

Sharding: the (batch x channel) space is split as 16 images x 24
channels per core (core = img_half * 4 + ch_quarter).  Versus pure
batch-parallel (4 img x 96 ch), this cuts the banded-weight DMA traffic
4x (each core loads 24 channels' W instead of 96) while x / y traffic
is unchanged -- the kernel is otherwise at the HBM roofline, with
weights 38% of bytes.  Channels are processed in DMA groups of G=4
(DRAM layout row-major [H, CH, ...]: one 6-15KB descriptor per
partition row).  Per channel: 5 dc x 4 img-chunk matmuls into 4 PSUM
banks -> scalar engine PSUM->SBUF fp16 copy with +(bias+1) fused ->
DVE tensor_tensor multiply by the center x (gate+residual
y = (conv*alpha + bias + 1) * x) -> grouped DMA out.
"""

import numpy as np
from contextlib import ExitStack

import concourse.bass as bass
import concourse.tile as tile
from concourse import bacc, mybir
from concourse.bass_utils import run_bass_kernel_spmd

F32 = mybir.dt.float32
FP16 = mybir.dt.float16
NPH = np.float16

B, H, W, CH, N = 32, 112, 112, 96, 5
NCORES = 8
IMG = 16                     # images per core
CPC = 24                     # channels per core
PAD = N // 2                 # 2
WP = W + 2 * PAD             # 116 padded cols per img
PIMG = 4                     # images per PSUM chunk
NP_ = IMG // PIMG            # img chunks (4)
PRE = 3                      # channel DMA prefetch depth


def _build_program():
    nc = bacc.Bacc("TRN2", target_bir_lowering=False, debug=False,
                   num_devices=NCORES)
    x_d = nc.dram_tensor("x", [H, CPC, IMG, WP], FP16,
                         kind="ExternalInput").ap()
    w_d = nc.dram_tensor("w", [H, CPC, N, H], FP16,
                         kind="ExternalInput").ap()
    cb_d = nc.dram_tensor("cb", [H, 1], F32, kind="ExternalInput").ap()
    y_d = nc.dram_tensor("y", [H, CPC, IMG, W], FP16,
                         kind="ExternalOutput").ap()

    with tile.TileContext(nc) as tc:
        _kernel(tc, y_d, x_d, w_d, cb_d)
    nc.compile()
    return nc


def _kernel(tc, y_d, x_d, w_d, cb_d):
    nc = tc.nc
    ctx = ExitStack()
    const_pool = ctx.enter_context(tc.tile_pool(name="const", bufs=1))
    x_pool = ctx.enter_context(tc.tile_pool(name="xp", bufs=PRE + 2))
    w_pool = ctx.enter_context(tc.tile_pool(name="wp", bufs=PRE + 2))
    ps_pool = ctx.enter_context(tc.tile_pool(name="ps", bufs=8, space="PSUM"))
    acc_pool = ctx.enter_context(tc.tile_pool(name="acc", bufs=3))

    cb = const_pool.tile([H, 1], F32)
    nc.sync.dma_start(out=cb[:], in_=cb_d[:, :])

    # PE warmup: TRN2's PE clock ramps 0.65 -> 1.2 -> 2.4 GHz over ~3us of
    # continuous activity.  Burn dummy matmuls on a zeroed scratch tile
    # while the first channel's DMA is in flight so real matmuls start at
    # full clock.
    warm = const_pool.tile([H, PIMG, W], FP16)
    nc.vector.memset(warm[:], 0.0)
    wps = ps_pool.tile([H, PIMG, W], F32, name="wps", tag="ps")
    for _ in range(12):
        nc.tensor.matmul(wps[:], lhsT=warm[:, 0, :], rhs=warm[:],
                         start=True, stop=True)

    def load(c):
        # per-channel DMAs keep dependency granularity fine: the first
        # matmul of channel c waits only on c's own (small) transfers.
        # The first channels trigger via the (empty) Activation queue so
        # their triggers aren't serialized behind SP-sequencer startup.
        eng = nc.scalar if c < 2 else nc.sync
        wt = w_pool.tile([H, N, H], FP16, name="wt", tag="wt")
        eng.dma_start(out=wt[:], in_=w_d[:, c])
        xt = x_pool.tile([H, IMG, WP], FP16, name="xt", tag="xt")
        if c == 0:
            # img-chunk pieces so the very first matmul starts ~0.3MB in
            for p in range(NP_):
                sl = slice(PIMG * p, PIMG * (p + 1))
                eng.dma_start(out=xt[:, sl], in_=x_d[:, c, sl])
        else:
            eng.dma_start(out=xt[:], in_=x_d[:, c])
        return xt, wt

    def conv(c, xt, wt):
        acc = acc_pool.tile([H, IMG, W], FP16, name="acc", tag="acc")
        ps = [ps_pool.tile([H, PIMG, W], F32, name="ps", tag="ps")
              for _ in range(NP_)]
        for dc in range(N):
            for p in range(NP_):
                nc.tensor.matmul(
                    ps[p][:], lhsT=wt[:, dc, :],
                    rhs=xt[:, PIMG * p:PIMG * (p + 1), dc:dc + W],
                    start=(dc == 0), stop=(dc == N - 1))
        for p in range(NP_):
            sl = slice(PIMG * p, PIMG * (p + 1))
            nc.scalar.add(out=acc[:, sl], in_=ps[p][:], add=cb[:, 0:1])
            nc.vector.tensor_mul(acc[:, sl], acc[:, sl],
                                 xt[:, sl, PAD:PAD + W])
        nc.sync.dma_start(out=y_d[:, c], in_=acc[:])

    live = {c: load(c) for c in range(PRE)}
    for c in range(CPC):
        if c + PRE < CPC:
            live[c + PRE] = load(c + PRE)
        xt, wt = live.pop(c)
        conv(c, xt, wt)
    ctx.close()


_prog_cache = {}


def _get_program():
    if "p" not in _prog_cache:
        _prog_cache["p"] = _build_program()
    return _prog_cache["p"]


def _prep_inputs(x, kernel, alpha, bias):
    x = np.asarray(x, dtype=np.float32)
    kernel = np.asarray(kernel, dtype=np.float32)
    a = float(np.asarray(alpha).reshape(-1)[0])
    b = float(np.asarray(bias).reshape(-1)[0])
    # x -> per-core [H, CPC, IMG, WP] fp16, cols zero-padded by 2.
    # core c = ih*4 + q handles imgs 16*ih.. and channels 24*q..
    xp = x.transpose(1, 3, 0, 2)                 # [H, CH, B, W]
    xr = np.zeros((NCORES, H, CPC, IMG, WP), dtype=NPH)
    xr[:, :, :, :, PAD:PAD + W] = (
        xp.reshape(H, 4, CPC, 2, IMG, W)          # [r, q, cc, ih, i, w]
        .transpose(3, 1, 0, 2, 4, 5)              # [ih, q, r, cc, i, w]
        .reshape(NCORES, H, CPC, IMG, W))
    # banded weights wt[i, c, dc, h] = a*k[i-h+2, dc, c] for |i-h|<=2
    wt = np.zeros((H, CH, N, H), dtype=np.float32)
    hh = np.arange(H)
    ak = a * kernel                                # [dr, dc, c]
    for dr in range(N):
        i = hh + dr - PAD
        m = (i >= 0) & (i < H)
        # advanced idx in dims 0,3 separated by slices -> result (nm, CH, N)
        wt[i[m], :, :, hh[m]] = ak[dr].T[None]
    wt = wt.astype(NPH)
    cb = np.full((H, 1), b + 1.0, dtype=np.float32)
    return xr, wt, cb


def _make_in_maps(xr, wt, cb):
    maps = []
    for c in range(NCORES):
        q = c % 4
        maps.append({"x": xr[c],
                     "w": np.ascontiguousarray(wt[:, CPC * q:CPC * (q + 1)]),
                     "cb": cb})
    return maps


def _gather(res):
    out = np.empty((B, H, W, CH), dtype=np.float32)
    for c in range(NCORES):
        ih, q = c // 4, c % 4
        y = np.asarray(res.results[c]["y"])        # [H, CPC, IMG, W]
        out[IMG * ih:IMG * (ih + 1), :, :, CPC * q:CPC * (q + 1)] = (
            y.transpose(2, 0, 3, 1).astype(np.float32))
    return out


def kernel(x, kernel, alpha, bias):
    xr, wt, cb = _prep_inputs(x, kernel, alpha, bias)
    nc = _get_program()
    res = run_bass_kernel_spmd(nc, _make_in_maps(xr, wt, cb),
                               list(range(NCORES)))
    return _gather(res)


# revision 13
# speedup vs baseline: 1.0117x; 1.0117x over previous
"""Trainium2 Bass kernel for ContourIntegrationLayer.

Reference computation (per batch element, fp32):
    conv = depthwise_conv2d(x, kernel, 5x5, SAME zero-pad)   # per-channel
    y    = (conv * alpha + bias) * x + x

Formulation: banded matmul over the ROW dimension.  Per channel c, x is
laid out with input rows on partitions and (img, col) on the free dim:
    xt[r, i, cp]  (112 part, img, 116 padded cols), fp16
The 5x5 depthwise conv becomes 5 accumulated PE matmul chains (one per
kernel column dc):
    out[h, (i,w)] = sum_dc sum_r  Wdc[r, h] * xt[r, i, w+dc]
where Wdc[r, h] = alpha*k[r-h+2, dc, c] for |r-h|<=2 (banded, built on
host, fp16).  K=112, M=112, N=448 per matmul -> 1 cycle/col in fp16:
2240 PE cycles per channel-image-set of 4 (the PE does 112 parallel
MACs/cycle along the contraction dim instead of a diag formulation's
1/lane).

Sharding: the (batch x channel) space is split as 16 images x 24
channels per core (core = img_half * 4 + ch_quarter).  Versus pure
batch-parallel (4 img x 96 ch), this cuts the banded-weight DMA traffic
4x (each core loads 24 channels' W instead of 96) while x / y traffic
is unchanged -- the kernel is otherwise at the HBM roofline, with
weights 38% of bytes.  Channels are processed in DMA groups of G=4
(DRAM layout row-major [H, CH, ...]: one 6-15KB descriptor per
partition row).  Per channel: 5 dc x 4 img-chunk matmuls into 4 PSUM
banks -> scalar engine PSUM->SBUF fp16 copy with +(bias+1) fused ->
DVE tensor_tensor multiply by the center x (gate+residual
y = (conv*alpha + bias + 1) * x) -> grouped DMA out.
"""

import numpy as np
from contextlib import ExitStack

import concourse.bass as bass
import concourse.tile as tile
from concourse import bacc, mybir
from concourse.bass_utils import run_bass_kernel_spmd

F32 = mybir.dt.float32
FP16 = mybir.dt.float16
NPH = np.float16

B, H, W, CH, N = 32, 112, 112, 96, 5
NCORES = 8
IMG = 16                     # images per core
CPC = 24                     # channels per core
PAD = N // 2                 # 2
WP = W + 2 * PAD             # 116 padded cols per img
PIMG = 4                     # images per PSUM chunk
NP_ = IMG // PIMG            # img chunks (4)
PRE = 3                      # channel DMA prefetch depth


def _build_program():
    nc = bacc.Bacc("TRN2", target_bir_lowering=False, debug=False,
                   num_devices=NCORES)
    x_d = nc.dram_tensor("x", [H, CPC, IMG, WP], FP16,
                         kind="ExternalInput").ap()
    w_d = nc.dram_tensor("w", [H, CPC, N, H], FP16,
                         kind="ExternalInput").ap()
    cb_d = nc.dram_tensor("cb", [H, 1], F32, kind="ExternalInput").ap()
    y_d = nc.dram_tensor("y", [H, CPC, IMG, W], FP16,
                         kind="ExternalOutput").ap()

    with tile.TileContext(nc) as tc:
        _kernel(tc, y_d, x_d, w_d, cb_d)
    nc.compile()
    return nc


def _kernel(tc, y_d, x_d, w_d, cb_d):
    nc = tc.nc
    ctx = ExitStack()
    const_pool = ctx.enter_context(tc.tile_pool(name="const", bufs=1))
    x_pool = ctx.enter_context(tc.tile_pool(name="xp", bufs=PRE + 2))
    w_pool = ctx.enter_context(tc.tile_pool(name="wp", bufs=PRE + 2))
    ps_pool = ctx.enter_context(tc.tile_pool(name="ps", bufs=8, space="PSUM"))
    acc_pool = ctx.enter_context(tc.tile_pool(name="acc", bufs=3))

    cb = const_pool.tile([H, 1], F32)
    nc.sync.dma_start(out=cb[:], in_=cb_d[:, :])

    # PE warmup: TRN2's PE clock ramps 0.65 -> 1.2 -> 2.4 GHz over ~3us of
    # continuous activity.  Burn dummy matmuls on a zeroed scratch tile
    # while the first channel's DMA is in flight so real matmuls start at
    # full clock.
    warm = const_pool.tile([H, PIMG, W], FP16)
    nc.vector.memset(warm[:], 0.0)
    wps = ps_pool.tile([H, PIMG, W], F32, name="wps", tag="ps")
    for _ in range(7):
        nc.tensor.matmul(wps[:], lhsT=warm[:, 0, :], rhs=warm[:],
                         start=True, stop=True)

    def load(c):
        # per-channel DMAs keep dependency granularity fine: the first
        # matmul of channel c waits only on c's own (small) transfers.
        wt = w_pool.tile([H, N, H], FP16, name="wt", tag="wt")
        nc.sync.dma_start(out=wt[:], in_=w_d[:, c])
        xt = x_pool.tile([H, IMG, WP], FP16, name="xt", tag="xt")
        if c == 0:
            # img-chunk pieces so the very first matmul starts ~0.3MB in
            for p in range(NP_):
                sl = slice(PIMG * p, PIMG * (p + 1))
                nc.sync.dma_start(out=xt[:, sl], in_=x_d[:, c, sl])
        else:
            nc.sync.dma_start(out=xt[:], in_=x_d[:, c])
        return xt, wt

    def conv(c, xt, wt):
        acc = acc_pool.tile([H, IMG, W], FP16, name="acc", tag="acc")
        ps = [ps_pool.tile([H, PIMG, W], F32, name="ps", tag="ps")
              for _ in range(NP_)]
        for dc in range(N):
            for p in range(NP_):
                nc.tensor.matmul(
                    ps[p][:], lhsT=wt[:, dc, :],
                    rhs=xt[:, PIMG * p:PIMG * (p + 1), dc:dc + W],
                    start=(dc == 0), stop=(dc == N - 1))
        for p in range(NP_):
            sl = slice(PIMG * p, PIMG * (p + 1))
            nc.scalar.add(out=acc[:, sl], in_=ps[p][:], add=cb[:, 0:1])
            nc.vector.tensor_mul(acc[:, sl], acc[:, sl],
                                 xt[:, sl, PAD:PAD + W])
        nc.sync.dma_start(out=y_d[:, c], in_=acc[:])

    live = {c: load(c) for c in range(PRE)}
    for c in range(CPC):
        if c + PRE < CPC:
            live[c + PRE] = load(c + PRE)
        xt, wt = live.pop(c)
        conv(c, xt, wt)
    ctx.close()


_prog_cache = {}


def _get_program():
    if "p" not in _prog_cache:
        _prog_cache["p"] = _build_program()
    return _prog_cache["p"]


def _prep_inputs(x, kernel, alpha, bias):
    x = np.asarray(x, dtype=np.float32)
    kernel = np.asarray(kernel, dtype=np.float32)
    a = float(np.asarray(alpha).reshape(-1)[0])
    b = float(np.asarray(bias).reshape(-1)[0])
    # x -> per-core [H, CPC, IMG, WP] fp16, cols zero-padded by 2.
    # core c = ih*4 + q handles imgs 16*ih.. and channels 24*q..
    xp = x.transpose(1, 3, 0, 2)                 # [H, CH, B, W]
    xr = np.zeros((NCORES, H, CPC, IMG, WP), dtype=NPH)
    xr[:, :, :, :, PAD:PAD + W] = (
        xp.reshape(H, 4, CPC, 2, IMG, W)          # [r, q, cc, ih, i, w]
        .transpose(3, 1, 0, 2, 4, 5)              # [ih, q, r, cc, i, w]
        .reshape(NCORES, H, CPC, IMG, W))
    # banded weights wt[i, c, dc, h] = a*k[i-h+2, dc, c] for |i-h|<=2
    wt = np.zeros((H, CH, N, H), dtype=np.float32)
    hh = np.arange(H)
    ak = a * kernel                                # [dr, dc, c]
    for dr in range(N):
        i = hh + dr - PAD
        m = (i >= 0) & (i < H)
        # advanced idx in dims 0,3 separated by slices -> result (nm, CH, N)
        wt[i[m], :, :, hh[m]] = ak[dr].T[None]
    wt = wt.astype(NPH)
    cb = np.full((H, 1), b + 1.0, dtype=np.float32)
    return xr, wt, cb


def _make_in_maps(xr, wt, cb):
    maps = []
    for c in range(NCORES):
        q = c % 4
        maps.append({"x": xr[c],
                     "w": np.ascontiguousarray(wt[:, CPC * q:CPC * (q + 1)]),
                     "cb": cb})
    return maps


def _gather(res):
    out = np.empty((B, H, W, CH), dtype=np.float32)
    for c in range(NCORES):
        ih, q = c // 4, c % 4
        y = np.asarray(res.results[c]["y"])        # [H, CPC, IMG, W]
        out[IMG * ih:IMG * (ih + 1), :, :, CPC * q:CPC * (q + 1)] = (
            y.transpose(2, 0, 3, 1).astype(np.float32))
    return out


def kernel(x, kernel, alpha, bias):
    xr, wt, cb = _prep_inputs(x, kernel, alpha, bias)
    nc = _get_program()
    res = run_bass_kernel_spmd(nc, _make_in_maps(xr, wt, cb),
                               list(range(NCORES)))
    return _gather(res)


# revision 14
# speedup vs baseline: 1.0162x; 1.0045x over previous
"""Trainium2 Bass kernel for ContourIntegrationLayer.

Reference computation (per batch element, fp32):
    conv = depthwise_conv2d(x, kernel, 5x5, SAME zero-pad)   # per-channel
    y    = (conv * alpha + bias) * x + x

Formulation: banded matmul over the ROW dimension.  Per channel c, x is
laid out with input rows on partitions and (img, col) on the free dim:
    xt[r, i, cp]  (112 part, img, 116 padded cols), fp16
The 5x5 depthwise conv becomes 5 accumulated PE matmul chains (one per
kernel column dc):
    out[h, (i,w)] = sum_dc sum_r  Wdc[r, h] * xt[r, i, w+dc]
where Wdc[r, h] = alpha*k[r-h+2, dc, c] for |r-h|<=2 (banded, built on
host, fp16).  K=112, M=112, N=448 per matmul -> 1 cycle/col in fp16:
2240 PE cycles per channel-image-set of 4 (the PE does 112 parallel
MACs/cycle along the contraction dim instead of a diag formulation's
1/lane).

Sharding: the (batch x channel) space is split as 16 images x 24
channels per core (core = img_half * 4 + ch_quarter).  Versus pure
batch-parallel (4 img x 96 ch), this cuts the banded-weight DMA traffic
4x (each core loads 24 channels' W instead of 96) while x / y traffic
is unchanged -- the kernel is otherwise at the HBM roofline, with
weights 38% of bytes.  Channels are processed in DMA groups of G=4
(DRAM layout row-major [H, CH, ...]: one 6-15KB descriptor per
partition row).  Per channel: 5 dc x 4 img-chunk matmuls into 4 PSUM
banks -> scalar engine PSUM->SBUF fp16 copy with +(bias+1) fused ->
DVE tensor_tensor multiply by the center x (gate+residual
y = (conv*alpha + bias + 1) * x) -> grouped DMA out.
"""

import numpy as np
from contextlib import ExitStack

import concourse.bass as bass
import concourse.tile as tile
from concourse import bacc, mybir
from concourse.bass_utils import run_bass_kernel_spmd

F32 = mybir.dt.float32
FP16 = mybir.dt.float16
NPH = np.float16

B, H, W, CH, N = 32, 112, 112, 96, 5
NCORES = 8
IMG = 16                     # images per core
CPC = 24                     # channels per core
PAD = N // 2                 # 2
WP = W + 2 * PAD             # 116 padded cols per img
PIMG = 4                     # images per PSUM chunk
NP_ = IMG // PIMG            # img chunks (4)
PRE = 3                      # channel DMA prefetch depth


def _build_program():
    nc = bacc.Bacc("TRN2", target_bir_lowering=False, debug=False,
                   num_devices=NCORES)
    x_d = nc.dram_tensor("x", [H, CPC, IMG, WP], FP16,
                         kind="ExternalInput").ap()
    w_d = nc.dram_tensor("w", [H, CPC, N, H], FP16,
                         kind="ExternalInput").ap()
    cb_d = nc.dram_tensor("cb", [H, 1], F32, kind="ExternalInput").ap()
    y_d = nc.dram_tensor("y", [H, CPC, IMG, W], FP16,
                         kind="ExternalOutput").ap()

    with tile.TileContext(nc) as tc:
        _kernel(tc, y_d, x_d, w_d, cb_d)
    nc.compile()
    return nc


def _kernel(tc, y_d, x_d, w_d, cb_d):
    nc = tc.nc
    ctx = ExitStack()
    const_pool = ctx.enter_context(tc.tile_pool(name="const", bufs=1))
    x_pool = ctx.enter_context(tc.tile_pool(name="xp", bufs=PRE + 2))
    w_pool = ctx.enter_context(tc.tile_pool(name="wp", bufs=PRE + 2))
    ps_pool = ctx.enter_context(tc.tile_pool(name="ps", bufs=8, space="PSUM"))
    acc_pool = ctx.enter_context(tc.tile_pool(name="acc", bufs=3))

    cb = const_pool.tile([H, 1], F32)
    nc.sync.dma_start(out=cb[:], in_=cb_d[:, :])

    def load(c):
        # per-channel DMAs keep dependency granularity fine: the first
        # matmul of channel c waits only on c's own (small) transfers.
        wt = w_pool.tile([H, N, H], FP16, name="wt", tag="wt")
        nc.sync.dma_start(out=wt[:], in_=w_d[:, c])
        xt = x_pool.tile([H, IMG, WP], FP16, name="xt", tag="xt")
        if c == 0:
            # img-chunk pieces so the very first matmul starts ~0.3MB in
            for p in range(NP_):
                sl = slice(PIMG * p, PIMG * (p + 1))
                nc.sync.dma_start(out=xt[:, sl], in_=x_d[:, c, sl])
        else:
            nc.sync.dma_start(out=xt[:], in_=x_d[:, c])
        return xt, wt

    def conv(c, xt, wt):
        acc = acc_pool.tile([H, IMG, W], FP16, name="acc", tag="acc")
        ps = [ps_pool.tile([H, PIMG, W], F32, name="ps", tag="ps")
              for _ in range(NP_)]
        for dc in range(N):
            for p in range(NP_):
                nc.tensor.matmul(
                    ps[p][:], lhsT=wt[:, dc, :],
                    rhs=xt[:, PIMG * p:PIMG * (p + 1), dc:dc + W],
                    start=(dc == 0), stop=(dc == N - 1))
        for p in range(NP_):
            sl = slice(PIMG * p, PIMG * (p + 1))
            nc.scalar.add(out=acc[:, sl], in_=ps[p][:], add=cb[:, 0:1])
            nc.vector.tensor_mul(acc[:, sl], acc[:, sl],
                                 xt[:, sl, PAD:PAD + W])
        nc.sync.dma_start(out=y_d[:, c], in_=acc[:])

    live = {c: load(c) for c in range(PRE)}
    for c in range(CPC):
        if c + PRE < CPC:
            live[c + PRE] = load(c + PRE)
        xt, wt = live.pop(c)
        conv(c, xt, wt)
    ctx.close()


_prog_cache = {}


def _get_program():
    if "p" not in _prog_cache:
        _prog_cache["p"] = _build_program()
    return _prog_cache["p"]


def _prep_inputs(x, kernel, alpha, bias):
    x = np.asarray(x, dtype=np.float32)
    kernel = np.asarray(kernel, dtype=np.float32)
    a = float(np.asarray(alpha).reshape(-1)[0])
    b = float(np.asarray(bias).reshape(-1)[0])
    # x -> per-core [H, CPC, IMG, WP] fp16, cols zero-padded by 2.
    # core c = ih*4 + q handles imgs 16*ih.. and channels 24*q..
    xp = x.transpose(1, 3, 0, 2)                 # [H, CH, B, W]
    xr = np.zeros((NCORES, H, CPC, IMG, WP), dtype=NPH)
    xr[:, :, :, :, PAD:PAD + W] = (
        xp.reshape(H, 4, CPC, 2, IMG, W)          # [r, q, cc, ih, i, w]
        .transpose(3, 1, 0, 2, 4, 5)              # [ih, q, r, cc, i, w]
        .reshape(NCORES, H, CPC, IMG, W))
    # banded weights wt[i, c, dc, h] = a*k[i-h+2, dc, c] for |i-h|<=2
    wt = np.zeros((H, CH, N, H), dtype=np.float32)
    hh = np.arange(H)
    ak = a * kernel                                # [dr, dc, c]
    for dr in range(N):
        i = hh + dr - PAD
        m = (i >= 0) & (i < H)
        # advanced idx in dims 0,3 separated by slices -> result (nm, CH, N)
        wt[i[m], :, :, hh[m]] = ak[dr].T[None]
    wt = wt.astype(NPH)
    cb = np.full((H, 1), b + 1.0, dtype=np.float32)
    return xr, wt, cb


def _make_in_maps(xr, wt, cb):
    maps = []
    for c in range(NCORES):
        q = c % 4
        maps.append({"x": xr[c],
                     "w": np.ascontiguousarray(wt[:, CPC * q:CPC * (q + 1)]),
                     "cb": cb})
    return maps


def _gather(res):
    out = np.empty((B, H, W, CH), dtype=np.float32)
    for c in range(NCORES):
        ih, q = c // 4, c % 4
        y = np.asarray(res.results[c]["y"])        # [H, CPC, IMG, W]
        out[IMG * ih:IMG * (ih + 1), :, :, CPC * q:CPC * (q + 1)] = (
            y.transpose(2, 0, 3, 1).astype(np.float32))
    return out


def kernel(x, kernel, alpha, bias):
    xr, wt, cb = _prep_inputs(x, kernel, alpha, bias)
    nc = _get_program()
    res = run_bass_kernel_spmd(nc, _make_in_maps(xr, wt, cb),
                               list(range(NCORES)))
    return _gather(res)


# revision 15
# speedup vs baseline: 1.0395x; 1.0229x over previous
"""Trainium2 Bass kernel for ContourIntegrationLayer.

Reference computation (per batch element, fp32):
    conv = depthwise_conv2d(x, kernel, 5x5, SAME zero-pad)   # per-channel
    y    = (conv * alpha + bias) * x + x

Formulation: banded matmul over the ROW dimension.  Per channel c, x is
laid out with input rows on partitions and (img, col) on the free dim:
    xt[r, i, cp]  (112 part, img, 116 padded cols), fp16
The 5x5 depthwise conv becomes 5 accumulated PE matmul chains (one per
kernel column dc):
    out[h, (i,w)] = sum_dc sum_r  Wdc[r, h] * xt[r, i, w+dc]
where Wdc[r, h] = alpha*k[r-h+2, dc, c] for |r-h|<=2 (banded, built on
host, fp16).  K=112, M=112, N=448 per matmul -> 1 cycle/col in fp16:
2240 PE cycles per channel-image-set of 4 (the PE does 112 parallel
MACs/cycle along the contraction dim instead of a diag formulation's
1/lane).

Sharding: the (batch x channel) space is split as 16 images x 24
channels per core (core = img_half * 4 + ch_quarter).  Versus pure
batch-parallel (4 img x 96 ch), this cuts the banded-weight DMA traffic
4x (each core loads 24 channels' W instead of 96) while x / y traffic
is unchanged -- the kernel is otherwise at the HBM roofline, with
weights 38% of bytes.  Channels are processed in DMA groups of G=4
(DRAM layout row-major [H, CH, ...]: one 6-15KB descriptor per
partition row).  Per channel: 5 dc x 4 img-chunk matmuls into 4 PSUM
banks -> scalar engine PSUM->SBUF fp16 copy with +(bias+1) fused ->
DVE tensor_tensor multiply by the center x (gate+residual
y = (conv*alpha + bias + 1) * x) -> grouped DMA out.
"""

import numpy as np
from contextlib import ExitStack

import concourse.bass as bass
import concourse.tile as tile
from concourse import bacc, mybir
from concourse.bass_utils import run_bass_kernel_spmd

F32 = mybir.dt.float32
FP16 = mybir.dt.float16
NPH = np.float16

B, H, W, CH, N = 32, 112, 112, 96, 5
NCORES = 8
IMG = 16                     # images per core
CPC = 24                     # channels per core
PAD = N // 2                 # 2
WP = W + 2 * PAD             # 116 padded cols per img
PIMG = 4                     # images per PSUM chunk
NP_ = IMG // PIMG            # img chunks (4)
PRE = 3                      # channel DMA prefetch depth


def _build_program():
    nc = bacc.Bacc("TRN2", target_bir_lowering=False, debug=False,
                   num_devices=NCORES)
    x_d = nc.dram_tensor("x", [H, CPC, IMG, WP], FP16,
                         kind="ExternalInput").ap()
    w_d = nc.dram_tensor("w", [H, CPC, N, H], FP16,
                         kind="ExternalInput").ap()
    cb_d = nc.dram_tensor("cb", [H, 1], F32, kind="ExternalInput").ap()
    y_d = nc.dram_tensor("y", [H, CPC, IMG, W], FP16,
                         kind="ExternalOutput").ap()

    with tile.TileContext(nc) as tc:
        _kernel(tc, y_d, x_d, w_d, cb_d)
    nc.compile()
    return nc


def _kernel(tc, y_d, x_d, w_d, cb_d):
    nc = tc.nc
    ctx = ExitStack()
    const_pool = ctx.enter_context(tc.tile_pool(name="const", bufs=1))
    x_pool = ctx.enter_context(tc.tile_pool(name="xp", bufs=PRE + 2))
    w_pool = ctx.enter_context(tc.tile_pool(name="wp", bufs=PRE + 2))
    ps_pool = ctx.enter_context(tc.tile_pool(name="ps", bufs=8, space="PSUM"))
    acc_pool = ctx.enter_context(tc.tile_pool(name="acc", bufs=3))

    cb = const_pool.tile([H, 1], F32)
    nc.sync.dma_start(out=cb[:], in_=cb_d[:, :])

    def load(c):
        # per-channel DMAs keep dependency granularity fine: the first
        # matmul of channel c waits only on c's own (small) transfers.
        wt = w_pool.tile([H, N, H], FP16, name="wt", tag="wt")
        nc.sync.dma_start(out=wt[:], in_=w_d[:, c])
        xt = x_pool.tile([H, IMG, WP], FP16, name="xt", tag="xt")
        if c == 0:
            # img-chunk pieces so the very first matmul starts ~0.3MB in
            for p in range(NP_):
                sl = slice(PIMG * p, PIMG * (p + 1))
                nc.sync.dma_start(out=xt[:, sl], in_=x_d[:, c, sl])
        else:
            nc.sync.dma_start(out=xt[:], in_=x_d[:, c])
        return xt, wt

    def conv(c, xt, wt):
        acc = acc_pool.tile([H, IMG, W], FP16, name="acc", tag="acc")
        last = c == CPC - 1
        ps = [ps_pool.tile([H, PIMG, W], F32, name="ps", tag="ps")
              for _ in range(NP_)]
        # steady state: dc-outer; last channel: chunk-outer so each img
        # chunk drains + stores while the PE finishes the rest (short tail)
        order = ([(p, dc) for p in range(NP_) for dc in range(N)] if last
                 else [(p, dc) for dc in range(N) for p in range(NP_)])
        for p, dc in order:
            nc.tensor.matmul(
                ps[p][:], lhsT=wt[:, dc, :],
                rhs=xt[:, PIMG * p:PIMG * (p + 1), dc:dc + W],
                start=(dc == 0), stop=(dc == N - 1))
        for p in range(NP_):
            sl = slice(PIMG * p, PIMG * (p + 1))
            nc.scalar.add(out=acc[:, sl], in_=ps[p][:], add=cb[:, 0:1])
            nc.vector.tensor_mul(acc[:, sl], acc[:, sl],
                                 xt[:, sl, PAD:PAD + W])
            if last:
                nc.sync.dma_start(out=y_d[:, c, sl], in_=acc[:, sl])
        if not last:
            nc.sync.dma_start(out=y_d[:, c], in_=acc[:])

    live = {c: load(c) for c in range(PRE)}
    for c in range(CPC):
        if c + PRE < CPC:
            live[c + PRE] = load(c + PRE)
        xt, wt = live.pop(c)
        conv(c, xt, wt)
    ctx.close()


_prog_cache = {}


def _get_program():
    if "p" not in _prog_cache:
        _prog_cache["p"] = _build_program()
    return _prog_cache["p"]


def _prep_inputs(x, kernel, alpha, bias):
    x = np.asarray(x, dtype=np.float32)
    kernel = np.asarray(kernel, dtype=np.float32)
    a = float(np.asarray(alpha).reshape(-1)[0])
    b = float(np.asarray(bias).reshape(-1)[0])
    # x -> per-core [H, CPC, IMG, WP] fp16, cols zero-padded by 2.
    # core c = ih*4 + q handles imgs 16*ih.. and channels 24*q..
    xp = x.transpose(1, 3, 0, 2)                 # [H, CH, B, W]
    xr = np.zeros((NCORES, H, CPC, IMG, WP), dtype=NPH)
    xr[:, :, :, :, PAD:PAD + W] = (
        xp.reshape(H, 4, CPC, 2, IMG, W)          # [r, q, cc, ih, i, w]
        .transpose(3, 1, 0, 2, 4, 5)              # [ih, q, r, cc, i, w]
        .reshape(NCORES, H, CPC, IMG, W))
    # banded weights wt[i, c, dc, h] = a*k[i-h+2, dc, c] for |i-h|<=2
    wt = np.zeros((H, CH, N, H), dtype=np.float32)
    hh = np.arange(H)
    ak = a * kernel                                # [dr, dc, c]
    for dr in range(N):
        i = hh + dr - PAD
        m = (i >= 0) & (i < H)
        # advanced idx in dims 0,3 separated by slices -> result (nm, CH, N)
        wt[i[m], :, :, hh[m]] = ak[dr].T[None]
    wt = wt.astype(NPH)
    cb = np.full((H, 1), b + 1.0, dtype=np.float32)
    return xr, wt, cb


def _make_in_maps(xr, wt, cb):
    maps = []
    for c in range(NCORES):
        q = c % 4
        maps.append({"x": xr[c],
                     "w": np.ascontiguousarray(wt[:, CPC * q:CPC * (q + 1)]),
                     "cb": cb})
    return maps


def _gather(res):
    out = np.empty((B, H, W, CH), dtype=np.float32)
    for c in range(NCORES):
        ih, q = c // 4, c % 4
        y = np.asarray(res.results[c]["y"])        # [H, CPC, IMG, W]
        out[IMG * ih:IMG * (ih + 1), :, :, CPC * q:CPC * (q + 1)] = (
            y.transpose(2, 0, 3, 1).astype(np.float32))
    return out


def kernel(x, kernel, alpha, bias):
    xr, wt, cb = _prep_inputs(x, kernel, alpha, bias)
    nc = _get_program()
    res = run_bass_kernel_spmd(nc, _make_in_maps(xr, wt, cb),
                               list(range(NCORES)))
    return _gather(res)
